# revision 1
# baseline (speedup 1.0000x reference)
"""Trainium2 Bass kernel for DFBNet SSP (sparse_attention).

Data-parallel over batch: 8 samples -> 8 NeuronCores, one sample per core.

Per-sample device computation (all heavy tensor work):
  - FP (masked avg-pool of support feat) and fg/bg prototypes of feature_q
  - column norms of feature_q, normalized cn
  - sim = 2 * cn.T @ cn                               [N,N] gram matmul
  - T[k,n] = wb[k] * exp(sim[k,n])  (additive -BIG mask fused into Exp bias)
  - colsum[n] = sum_k T[k,n] (== softmax row-sums by symmetry of sim)
  - bg_local[c,n] = sum_k fq[c,k] T[k,n] / colsum[n]  (== (bg_attn @ cur.T).T)
  - BP1 ~ bg_proto*(3/7) + bg_local, FP1 ~ FP + fg_proto (cosine is
    scale-invariant so the reference's 0.3/0.7 and 0.5/0.5 blends are applied
    up to a positive scale that cancels)
  - out = 10 * cosine(feature_q, {BP1, FP1}) along C

Host side computes only the {0,1} threshold-selection vectors wf/wb (float64
replica of the reference pred chain incl. the top-k fallback).  These are
discrete bits whose exact values a device fp32 pipeline could flip at
~1e-7-margin pixels, with O(1) output impact; everything continuous stays on
device.
"""

import numpy as np

B, C, H, W = 8, 512, 32, 32
N = H * W
FG_THRES, BG_THRES, TOPK = 0.7, 0.6, 12
BIG = 60000.0
LN10 = 2.302585092994046  # additive pre-exp mask; exp(x - BIG) == 0.0 in fp32

CC = C // 128  # 4 channel chunks
KC = N // 128  # 8 pixel chunks
NB = N // 512  # 2 psum-bank column groups

_cache = {}
_EYE = np.eye(128, dtype=np.float32)


# --------------------------------------------------------------------------
# host: selection weights (exact reference semantics, float64)
# --------------------------------------------------------------------------
def _host_select_weights(feature_q, support_feat, support_mask):
    fq = feature_q.astype(np.float64).reshape(B, C, N)
    sf = support_feat.astype(np.float64).reshape(B, C, N)
    mf = (support_mask.reshape(B, N) == 1).astype(np.float64)
    mb = 1.0 - mf
    FP = (sf * mf[:, None]).sum(-1) / (mf.sum(-1)[:, None] + 1e-5)
    BP = (sf * mb[:, None]).sum(-1) / (mb.sum(-1)[:, None] + 1e-5)

    def cos(a, b):  # a [B,C,N], b [B,C]
        dot = (a * b[:, :, None]).sum(1)
        na = np.sqrt((a * a).sum(1))
        nb = np.sqrt((b * b).sum(1))[:, None]
        return dot / np.maximum(na * nb, 1e-8)

    sfg = cos(fq, FP) * 10.0
    sbg = cos(fq, BP) * 10.0
    m = np.maximum(sfg, sbg)
    efg = np.exp(sfg - m)
    ebg = np.exp(sbg - m)
    pfg = efg / (efg + ebg)
    pbg = ebg / (efg + ebg)

    def select(pred, thres):
        w = np.zeros((B, N), np.float32)
        for b in range(B):
            row = pred[b] > thres
            if row.sum() > 0:
                w[b] = row
            else:
                # jax.lax.top_k tie-break: lower index wins -> stable argsort
                idx = np.argsort(-pred[b], kind="stable")[:TOPK]
                w[b, idx] = 1.0
        return w

    return select(pfg, FG_THRES), select(pbg, BG_THRES)


# --------------------------------------------------------------------------
# device program
# --------------------------------------------------------------------------
def _make_tile_context_cls():
    import concourse.tile as tile
    from concourse.vector_clock import ScopedClock, VectorClock

    class PatchedTileContext(tile.TileContext):
        """This walrus build rejects CTRL/Drain instructions carrying more
        than one sem wait.  Put the tail-drain's global-clock waits on
        single-wait NOPs (same engine, program order) instead."""

        def _drain_and_barrier(self, tick_clock, wait_clock):
            gc = tick_clock.global_clock
            n = len(gc)
            for proc in range(n):
                t = gc[proc]
                if t > 0:
                    vec = [0] * n
                    vec[proc] = t
                    nop = self.nc.sync.nop(nofuse=True)
                    wait_clock.add_sem_waits(
                        nop.ins, ScopedClock({None: VectorClock(vec)})
                    )
            self.nc.sync.drain()
            self.nc.all_engine_barrier()
            assert self.sems is not None
            popped = self.nc._tile_sem_poison_stack.pop()
            assert popped is self._sem_poison
            self.nc.clear_and_free_semaphores(list(self.sems.allocated().values()))
            self.nc.all_engine_barrier()

    return PatchedTileContext


def _split_multi_waits(nc):
    """This walrus build allows at most one sync-wait command per
    instruction.  Move extra waits onto same-engine NOPs inserted just
    before the instruction (waits are AND conditions; order-safe)."""
    import concourse.mybir as mybir

    n_split = 0
    for f in nc.m.functions:
        for bb in f.blocks:
            il = bb.instructions
            i = 0
            while i < len(il):
                inst = il[i]
                si = inst.sync_info
                if si is not None and si.on_wait and len(si.on_wait) > 1:
                    waits = list(si.on_wait)
                    for j, w in enumerate(waits[:-1]):
                        nop = mybir.InstNoOp(
                            name=f"{inst.name}-wsplit{j}",
                            ins=[],
                            outs=[],
                            engine=inst.engine,
                            sync_info=mybir.SyncInfo(on_wait=[w], on_update=[]),
                        )
                        il.insert(i, nop)
                        i += 1
                        n_split += 1
                    inst.sync_info = mybir.SyncInfo(
                        on_wait=[waits[-1]], on_update=si.on_update
                    )
                i += 1
    return n_split


def _build_nc(split_waits=True):
    import concourse.bass as bass
    import concourse.mybir as mybir

    fp32 = mybir.dt.float32
    f32r = mybir.dt.float32r
    AF = mybir.ActivationFunctionType
    ALU = mybir.AluOpType
    AX = mybir.AxisListType

    PatchedTileContext = _make_tile_context_cls()

    nc = bass.Bass("TRN2", target_bir_lowering=False)
    fq_d = nc.declare_dram_parameter("fq", [C, N], fp32, isOutput=False)
    id_d = nc.declare_dram_parameter("ident", [128, 128], fp32, isOutput=False)
    sf_d = nc.declare_dram_parameter("sf", [C, N], fp32, isOutput=False)
    mf_d = nc.declare_dram_parameter("mf", [1, N], fp32, isOutput=False)
    wf_d = nc.declare_dram_parameter("wf", [1, N], fp32, isOutput=False)
    wb_d = nc.declare_dram_parameter("wb", [1, N], fp32, isOutput=False)
    out_d = nc.declare_dram_parameter("out", [2, N], fp32, isOutput=True)

    def nbs(nb):
        return slice(nb * 512, (nb + 1) * 512)

    with PatchedTileContext(nc) as tc:
        with (
            tc.tile_pool(name="consts", bufs=1) as consts,
            tc.tile_pool(name="big", bufs=1) as big,
            tc.tile_pool(name="scr", bufs=2) as scr,
            tc.tile_pool(name="small", bufs=1) as small,
        ):
            # ---- constants / small inputs
            ident = consts.tile([128, 128], fp32, tag="ident")
            nc.sync.dma_start(ident, id_d[:, :])
            ones_f = consts.tile([128, 128], fp32, tag="ones_f")
            nc.vector.memset(ones_f, 1.0)
            ones = consts.tile([128, 128], f32r, tag="ones")
            nc.vector.tensor_copy(ones, ones_f)

            ln10c = consts.tile([1, 1], fp32, tag="ln10c")
            nc.vector.memset(ln10c, LN10)
            mfrow = consts.tile([1, N], fp32, tag="mfrow")
            nc.sync.dma_start(mfrow, mf_d[:, :])
            wfrow = consts.tile([1, N], fp32, tag="wfrow")
            nc.sync.dma_start(wfrow, wf_d[:, :])
            wbrow = consts.tile([1, N], fp32, tag="wbrow")
            nc.sync.dma_start(wbrow, wb_d[:, :])
            wbcol = consts.tile([128, KC], fp32, tag="wbcol")
            nc.sync.dma_start(wbcol, wb_d[0, :].rearrange("(a b) -> b a", b=128))
            # bias = (wb - 1) * BIG  ->  {0 -> -BIG, 1 -> 0}
            biascol = consts.tile([128, KC], fp32, tag="biascol")
            nc.vector.tensor_scalar(
                biascol, wbcol, BIG, BIG, op0=ALU.mult, op1=ALU.subtract
            )

            # ---- main inputs
            fq = []
            sfc = []
            for cc in range(CC):
                t = big.tile([128, N], fp32, tag=f"fq{cc}", name=f"fqs{cc}")
                nc.sync.dma_start(t, fq_d[cc * 128 : (cc + 1) * 128, :])
                fq.append(t)
            for cc in range(CC):
                t = big.tile([128, N], fp32, tag=f"sf{cc}", name=f"sfs{cc}")
                nc.sync.dma_start(t, sf_d[cc * 128 : (cc + 1) * 128, :])
                sfc.append(t)

            # f32r row copies (broadcast matmul operands; 0/1 exact in f32r)
            mfrow_r = consts.tile([1, N], f32r, tag="mfrow_r")
            nc.vector.tensor_copy(mfrow_r, mfrow)
            wfrow_r = consts.tile([1, N], f32r, tag="wfrow_r")
            nc.vector.tensor_copy(wfrow_r, wfrow)
            wbrow_r = consts.tile([1, N], f32r, tag="wbrow_r")
            nc.vector.tensor_copy(wbrow_r, wbrow)
            # ---- mask broadcasts [128, N] via K=1 ones-matmul (PSUM) + copy
            mfB = consts.tile([128, N], fp32, tag="mfB")
            wfB = consts.tile([128, N], fp32, tag="wfB")
            wbB = consts.tile([128, N], fp32, tag="wbB")

            # ---- transposes (PE) + column norms
            fqT = [big.tile([128, C], f32r, tag=f"fqT{kc}", name=f"fqT{kc}") for kc in range(KC)]
            na2row = consts.tile([1, N], fp32, tag="na2row")
            rnormB = big.tile([128, N], fp32, tag="rnormB")
            with tc.tile_pool(name="ps_pre", bufs=2, space="PSUM") as ps_pre:
                for row, dst in ((mfrow_r, mfB), (wfrow_r, wfB), (wbrow_r, wbB)):
                    for nb in range(NB):
                        bc = ps_pre.tile([128, 512], fp32, tag="bc", name="bc")
                        nc.tensor.matmul(
                            bc, ones[0:1, :], row[:, nbs(nb)], start=True, stop=True
                        )
                        nc.scalar.copy(dst[:, nbs(nb)], bc)
                for kc in range(KC):
                    trp = ps_pre.tile([128, 512], fp32, tag="tr", name=f"trp{kc}")
                    for cc in range(CC):
                        nc.tensor.transpose(
                            trp[:, cc * 128 : (cc + 1) * 128],
                            fq[cc][:, kc * 128 : (kc + 1) * 128],
                            ident,
                        )
                    nc.scalar.copy(fqT[kc], trp)

                n2ps = [ps_pre.tile([128, 512], fp32, tag="n2", name=f"n2ps{nb}") for nb in range(NB)]
                for cc in range(CC):
                    sq = scr.tile([128, N], f32r, tag="sqr", bufs=2, name="sq")
                    nc.vector.tensor_mul(sq, fq[cc], fq[cc])
                    for nb in range(NB):
                        nc.tensor.matmul(
                            n2ps[nb],
                            ones,
                            sq[:, nbs(nb)],
                            start=(cc == 0),
                            stop=(cc == CC - 1),
                        )
                tmp = scr.tile([128, N], fp32, tag="scr")
                for nb in range(NB):
                    nc.vector.tensor_copy(na2row[:, nbs(nb)], n2ps[nb][0:1, :])
                    nc.scalar.activation(tmp[:, nbs(nb)], n2ps[nb], AF.Ln)
                nc.scalar.activation(rnormB, tmp, AF.Exp, scale=-0.5)

            # ---- cn = fq * rnormB
            cn = []
            for cc in range(CC):
                t = big.tile([128, N], f32r, tag=f"cn{cc}", name=f"cns{cc}")
                nc.vector.tensor_mul(t, fq[cc], rnormB)
                cn.append(t)

            # ---- prototypes (free-dim masked reductions on DVE)
            FPr = small.tile([128, CC], fp32, tag="FPr")
            FGr = small.tile([128, CC], fp32, tag="FGr")
            BGr = small.tile([128, CC], fp32, tag="BGr")
            # gpsimd is otherwise idle and these are off the critical path
            for cc in range(CC):
                for acc, a, b in (
                    (FPr, sfc[cc], mfB),
                    (FGr, fq[cc], wfB),
                    (BGr, fq[cc], wbB),
                ):
                    o = scr.tile([128, N], fp32, tag="gscr", bufs=2, name="ttro")
                    nc.gpsimd.tensor_mul(o, a, b)
                    snk = scr.tile([128, N], fp32, tag="scr", name="snk")
                    nc.scalar.activation(
                        snk, o, AF.Copy, accum_out=acc[:, cc : cc + 1]
                    )
            cntm = small.tile([128, 1], fp32, tag="cntm")
            nc.vector.reduce_sum(cntm, mfB, axis=AX.X)
            cntf = small.tile([128, 1], fp32, tag="cntf")
            nc.vector.reduce_sum(cntf, wfB, axis=AX.X)
            cntb = small.tile([128, 1], fp32, tag="cntb")
            nc.vector.reduce_sum(cntb, wbB, axis=AX.X)

            rcntm = small.tile([128, 1], fp32, tag="rcntm")
            nc.vector.tensor_scalar_add(rcntm, cntm, 1e-5)
            nc.vector.reciprocal(rcntm, rcntm)
            rcntf = small.tile([128, 1], fp32, tag="rcntf")
            nc.vector.reciprocal(rcntf, cntf)
            rcntb = small.tile([128, 1], fp32, tag="rcntb")
            nc.vector.reciprocal(rcntb, cntb)
            nc.vector.tensor_scalar_mul(rcntb, rcntb, 3.0 / 7.0)

            # FP1 ~ FP + fg_proto  (2*FP_1 of the reference; scale cancels)
            FP1 = small.tile([128, CC], fp32, tag="FP1")
            nc.vector.tensor_scalar_mul(FP1, FPr, rcntm)
            tmp4 = small.tile([128, CC], fp32, tag="tmp4")
            nc.vector.tensor_scalar_mul(tmp4, FGr, rcntf)
            nc.vector.tensor_add(FP1, FP1, tmp4)
            # bgp_s = (3/7) * bg_proto
            bgp_s = small.tile([128, CC], fp32, tag="bgp_s")
            nc.vector.tensor_scalar_mul(bgp_s, BGr, rcntb)

            # ---- gram + exp + colsum + bg reconstruction
            T = [big.tile([128, N], f32r, tag=f"T{kc}", name=f"T{kc}") for kc in range(KC)]
            rcolB = big.tile([128, N], fp32, tag="rcolB")
            BP1 = [big.tile([128, N], fp32, tag=f"BP1{cc}", name=f"BP1_{cc}") for cc in range(CC)]
            with (
                tc.tile_pool(name="ps_sim", bufs=4, space="PSUM") as ps_sim,
                tc.tile_pool(name="ps_cs", bufs=2, space="PSUM") as ps_cs,
                tc.tile_pool(name="ps_bg", bufs=2, space="PSUM") as ps_bg,
            ):
                csps = [ps_cs.tile([128, 512], fp32, tag="cs", name=f"csps{nb}") for nb in range(NB)]
                for mi in range(KC):
                    for nb in range(NB):
                        simp = ps_sim.tile([128, 512], fp32, tag="sim", name=f"simp{mi}_{nb}")
                        for cc in range(CC):
                            nc.tensor.matmul(
                                simp,
                                cn[cc][:, mi * 128 : (mi + 1) * 128],
                                cn[cc][:, nbs(nb)],
                                start=(cc == 0),
                                stop=(cc == CC - 1),
                            )
                        nc.scalar.activation(
                            T[mi][:, nbs(nb)],
                            simp,
                            AF.Exp,
                            bias=biascol[:, mi : mi + 1],
                            scale=2.0,
                        )
                        nc.tensor.matmul(
                            csps[nb],
                            ones,
                            T[mi][:, nbs(nb)],
                            start=(mi == 0),
                            stop=(mi == KC - 1),
                        )
                tmpc = scr.tile([128, N], fp32, tag="scr")
                for nb in range(NB):
                    nc.scalar.activation(tmpc[:, nbs(nb)], csps[nb], AF.Ln)
                nc.scalar.activation(rcolB, tmpc, AF.Exp, scale=-1.0)

                for mi2 in range(CC):
                    bgp = [ps_bg.tile([128, 512], fp32, tag="bg", name=f"bgp{mi2}_{nb}") for nb in range(NB)]
                    for kc in range(KC):
                        for nb in range(NB):
                            nc.tensor.matmul(
                                bgp[nb],
                                fqT[kc][:, mi2 * 128 : (mi2 + 1) * 128],
                                T[kc][:, nbs(nb)],
                                start=(kc == 0),
                                stop=(kc == KC - 1),
                            )
                    for nb in range(NB):
                        nc.vector.tensor_mul(
                            BP1[mi2][:, nbs(nb)], bgp[nb], rcolB[:, nbs(nb)]
                        )
                    nc.vector.tensor_scalar_add(
                        BP1[mi2], BP1[mi2], bgp_s[:, mi2 : mi2 + 1]
                    )

            # ---- final similarities
            with tc.tile_pool(name="ps_fin", bufs=1, space="PSUM") as ps_fin:
                dfg = [ps_fin.tile([1, 512], fp32, tag=f"dfg{nb}", name=f"dfg{nb}") for nb in range(NB)]
                for cc in range(CC):
                    for nb in range(NB):
                        nc.tensor.matmul(
                            dfg[nb],
                            FP1[:, cc : cc + 1],
                            fq[cc][:, nbs(nb)],
                            start=(cc == 0),
                            stop=(cc == CC - 1),
                        )
                sqf = small.tile([128, CC], fp32, tag="sqf")
                nc.vector.tensor_mul(sqf, FP1, FP1)
                rsum = small.tile([128, 1], fp32, tag="rsum")
                nc.vector.reduce_sum(rsum, sqf, axis=AX.X)
                nfps = ps_fin.tile([1, 1], fp32, tag="nfp2")
                nc.tensor.matmul(nfps, ones_f[:, 0:1], rsum, start=True, stop=True)
                nfp2s = small.tile([1, 1], fp32, tag="nfp2s")
                nc.vector.tensor_copy(nfp2s, nfps)

                dbg = [ps_fin.tile([1, 512], fp32, tag=f"dbg{nb}", name=f"dbg{nb}") for nb in range(NB)]
                qps = [ps_fin.tile([1, 512], fp32, tag=f"q{nb}", name=f"qps{nb}") for nb in range(NB)]
                for cc in range(CC):
                    p_t = scr.tile([128, N], f32r, tag="sqr", bufs=2, name="p_t")
                    nc.vector.tensor_mul(p_t, fq[cc], BP1[cc])
                    q_t = scr.tile([128, N], f32r, tag="sqr", bufs=2, name="q_t")
                    nc.vector.tensor_mul(q_t, BP1[cc], BP1[cc])
                    for nb in range(NB):
                        nc.tensor.matmul(
                            dbg[nb],
                            ones[:, 0:1],
                            p_t[:, nbs(nb)],
                            start=(cc == 0),
                            stop=(cc == CC - 1),
                        )
                        nc.tensor.matmul(
                            qps[nb],
                            ones[:, 0:1],
                            q_t[:, nbs(nb)],
                            start=(cc == 0),
                            stop=(cc == CC - 1),
                        )

                # final rows: two separate [1,N] chains (partition 0 only)
                dotfg_s = small.tile([1, N], fp32, tag="rowtmp", bufs=5, name="dotfg_s")
                for nb in range(NB):
                    nc.vector.tensor_copy(dotfg_s[:, nbs(nb)], dfg[nb])
                prodfg = small.tile([1, N], fp32, tag="rowtmp", bufs=5, name="prodfg")
                nc.scalar.mul(prodfg, na2row, nfp2s)
                nc.vector.tensor_scalar(prodfg, prodfg, 1e-16, None, op0=ALU.max)
                nc.scalar.activation(prodfg, prodfg, AF.Ln)
                nc.scalar.activation(prodfg, prodfg, AF.Exp, scale=-0.5, bias=ln10c)
                outfg = small.tile([1, N], fp32, tag="rowtmp", bufs=5, name="outfg")
                nc.vector.tensor_mul(outfg, dotfg_s, prodfg)
                nc.sync.dma_start(out_d[1:2, :], outfg)

                dotbg_s = small.tile([1, N], fp32, tag="rowtmp", bufs=5, name="dotbg_s")
                nb2bg = small.tile([1, N], fp32, tag="rowtmp", bufs=5, name="nb2bg")
                for nb in range(NB):
                    nc.vector.tensor_copy(dotbg_s[:, nbs(nb)], dbg[nb])
                    nc.vector.tensor_copy(nb2bg[:, nbs(nb)], qps[nb])
                prodbg = small.tile([1, N], fp32, tag="rowtmp", bufs=5, name="prodbg")
                nc.vector.tensor_mul(prodbg, na2row, nb2bg)
                nc.vector.tensor_scalar(prodbg, prodbg, 1e-16, None, op0=ALU.max)
                nc.scalar.activation(prodbg, prodbg, AF.Ln)
                nc.scalar.activation(prodbg, prodbg, AF.Exp, scale=-0.5, bias=ln10c)
                outbg = small.tile([1, N], fp32, tag="rowtmp", bufs=5, name="outbg")
                nc.vector.tensor_mul(outbg, dotbg_s, prodbg)
                nc.sync.dma_start(out_d[0:1, :], outbg)

    if split_waits:
        _split_multi_waits(nc)
    return nc


def _get_nc():
    if "nc" not in _cache:
        _cache["nc"] = _build_nc()
    return _cache["nc"]


def _make_in_maps(feature_q, support_feat, support_mask):
    wf, wb = _host_select_weights(feature_q, support_feat, support_mask)
    fqr = np.ascontiguousarray(feature_q.reshape(B, C, N), dtype=np.float32)
    sfr = np.ascontiguousarray(support_feat.reshape(B, C, N), dtype=np.float32)
    mfr = (support_mask.reshape(B, N) == 1).astype(np.float32)
    return [
        {
            "fq": fqr[b],
            "ident": _EYE,
            "sf": sfr[b],
            "mf": mfr[b : b + 1],
            "wf": wf[b : b + 1],
            "wb": wb[b : b + 1],
        }
        for b in range(B)
    ]


def run_sharded(feature_q, support_feat, support_mask, **kwargs):
    """Run on all 8 cores; returns (output [B,2,H,W], BassKernelResults)."""
    from concourse.bass_utils import run_bass_kernel_spmd

    nc = _get_nc()
    in_maps = _make_in_maps(feature_q, support_feat, support_mask)
    res = run_bass_kernel_spmd(nc, in_maps, core_ids=list(range(B)), **kwargs)
    out = np.stack([res.results[b]["out"] for b in range(B)])
    return out.reshape(B, 2, H, W).astype(np.float32), res


def kernel(feature_q, support_feat, support_mask):
    out, _ = run_sharded(
        np.asarray(feature_q), np.asarray(support_feat), np.asarray(support_mask)
    )
    return out



# revision 7
# speedup vs baseline: 2.2781x; 2.2781x over previous
"""Trainium2 Bass kernel for DFBNet SSP (sparse_attention).

Data-parallel over batch: 8 samples -> 8 NeuronCores, one sample per core.

Sparse formulation: the reference's bg softmax masks to the wb-active columns
(|wb| ~ 270-320 of N=1024), so the [N,N] gram is really [KB,N] with KB the
padded active count.  The host computes the discrete {0,1} selection vectors
(exact fp64 replica of the reference pred chain incl. top-k fallback), turns
them into index gathers of the bf16-rounded inputs, and ships:

  fq    [C, N]   bf16   full features (sim rhs, norms)
  ga    [C, GW]  bf16   [fqa | fqf | sfm] gathered cols, zero-padded
  gt    [KB, C]  bf16   fqa transposed (recon stationary)
  sm    [128, 3+MI] f32 wb-active indicator (chunk layout) + 1/count scalars
  wp    [1, KB]  bf16   pad indicator row (fixes norms of zero pad cols)

Device (per core) computes everything continuous:
  norms + cn = fq/||fq||, cna; sim = cna^T cn [KB,N]; T = wb*exp(2 sim)
  (additive -BIG mask in the Exp bias); colsum via ones-matmul; rcol = 1/cs;
  T' = T*rcol; recon = fqa @ T' (= bg_local); BP1 = recon + (3/7)bg_proto;
  out0 = 10*cos(fq,BP1) via cn-folded dots; FP1 = FP + fg_proto (cosine
  scale-invariance drops the reference's 0.5/0.5 and 0.3/0.7 blend scales);
  out1 = (FP1*10/||FP1||)^T cn.
"""

import numpy as np
import ml_dtypes

B, C, H, W = 8, 512, 32, 32
N = H * W
CC = C // 128  # 4 channel chunks
FG_THRES, BG_THRES, TOPK = 0.7, 0.6, 12
BIG = 60000.0

# default gather capacities (multiples: KB of 128; KF/KM of 64)
KB0, KF0, KM0 = 384, 256, 640

_cache = {}


# --------------------------------------------------------------------------
# host: selection weights (exact reference semantics, float64)
# --------------------------------------------------------------------------
def _host_select_weights(feature_q, support_feat, support_mask):
    fq = feature_q.astype(np.float64).reshape(B, C, N)
    sf = support_feat.astype(np.float64).reshape(B, C, N)
    mf = (support_mask.reshape(B, N) == 1).astype(np.float64)
    mb = 1.0 - mf
    FP = (sf * mf[:, None]).sum(-1) / (mf.sum(-1)[:, None] + 1e-5)
    BP = (sf * mb[:, None]).sum(-1) / (mb.sum(-1)[:, None] + 1e-5)

    def cos(a, b):  # a [B,C,N], b [B,C]
        dot = (a * b[:, :, None]).sum(1)
        na = np.sqrt((a * a).sum(1))
        nb = np.sqrt((b * b).sum(1))[:, None]
        return dot / np.maximum(na * nb, 1e-8)

    sfg = cos(fq, FP) * 10.0
    sbg = cos(fq, BP) * 10.0
    m = np.maximum(sfg, sbg)
    efg = np.exp(sfg - m)
    ebg = np.exp(sbg - m)
    pfg = efg / (efg + ebg)
    pbg = ebg / (efg + ebg)

    def select(pred, thres):
        w = np.zeros((B, N), np.float32)
        for b in range(B):
            row = pred[b] > thres
            if row.sum() > 0:
                w[b] = row
            else:
                # jax.lax.top_k tie-break: lower index wins -> stable argsort
                idx = np.argsort(-pred[b], kind="stable")[:TOPK]
                w[b, idx] = 1.0
        return w

    return select(pfg, FG_THRES), select(pbg, BG_THRES)


# --------------------------------------------------------------------------
# build-environment workarounds (this walrus build's sync-wait limits)
# --------------------------------------------------------------------------
def _make_tile_context_cls():
    import concourse.tile as tile
    from concourse.vector_clock import ScopedClock, VectorClock

    class PatchedTileContext(tile.TileContext):
        """This walrus build rejects CTRL/Drain instructions carrying more
        than one sem wait.  Put the tail-drain's global-clock waits on
        single-wait NOPs (same engine, program order) instead."""

        def _drain_and_barrier(self, tick_clock, wait_clock):
            gc = tick_clock.global_clock
            n = len(gc)
            for proc in range(n):
                t = gc[proc]
                if t > 0:
                    vec = [0] * n
                    vec[proc] = t
                    nop = self.nc.sync.nop(nofuse=True)
                    wait_clock.add_sem_waits(
                        nop.ins, ScopedClock({None: VectorClock(vec)})
                    )
            self.nc.sync.drain()
            self.nc.all_engine_barrier()
            assert self.sems is not None
            popped = self.nc._tile_sem_poison_stack.pop()
            assert popped is self._sem_poison
            self.nc.clear_and_free_semaphores(list(self.sems.allocated().values()))
            self.nc.all_engine_barrier()

    return PatchedTileContext


def _split_multi_waits(nc):
    """This walrus build allows at most one sync-wait command per
    instruction.  Move extra waits onto same-engine NOPs inserted just
    before the instruction (waits are AND conditions; order-safe)."""
    import concourse.mybir as mybir

    n_split = 0
    for f in nc.m.functions:
        for bb in f.blocks:
            il = bb.instructions
            i = 0
            while i < len(il):
                inst = il[i]
                si = inst.sync_info
                if si is not None and si.on_wait and len(si.on_wait) > 1:
                    waits = list(si.on_wait)
                    for j, w in enumerate(waits[:-1]):
                        nop = mybir.InstNoOp(
                            name=f"{inst.name}-wsplit{j}",
                            ins=[],
                            outs=[],
                            engine=inst.engine,
                            sync_info=mybir.SyncInfo(on_wait=[w], on_update=[]),
                        )
                        il.insert(i, nop)
                        i += 1
                        n_split += 1
                    inst.sync_info = mybir.SyncInfo(
                        on_wait=[waits[-1]], on_update=si.on_update
                    )
                i += 1
    return n_split


# --------------------------------------------------------------------------
# device program
# --------------------------------------------------------------------------
def _build_nc(KB, KF, KM, split_waits=True):
    import concourse.bass as bass
    import concourse.mybir as mybir

    fp32 = mybir.dt.float32
    bf16 = mybir.dt.bfloat16
    AF = mybir.ActivationFunctionType
    ALU = mybir.AluOpType
    AX = mybir.AxisListType

    MI = KB // 128  # active-row chunks
    GW = KB + KF + KM  # packed gather width per channel chunk
    SMW = MI + 5

    PatchedTileContext = _make_tile_context_cls()

    nc = bass.Bass("TRN2", target_bir_lowering=False)
    fq_d = nc.declare_dram_parameter("fq", [C, N], bf16, isOutput=False)
    ga_d = nc.declare_dram_parameter("ga", [C, GW], bf16, isOutput=False)
    gt_d = nc.declare_dram_parameter("gt", [KB, C], bf16, isOutput=False)
    sm_d = nc.declare_dram_parameter("sm", [128, SMW], fp32, isOutput=False)
    wp_d = nc.declare_dram_parameter("wp", [1, KB], bf16, isOutput=False)
    out_d = nc.declare_dram_parameter("out", [2, N], fp32, isOutput=True)

    def nbs(nb):
        return slice(nb * 512, (nb + 1) * 512)

    def c128(cc):
        return slice(cc * 128, (cc + 1) * 128)

    with PatchedTileContext(nc) as tc:
        with (
            tc.tile_pool(name="consts", bufs=1) as consts,
            tc.tile_pool(name="big", bufs=1) as big,
            tc.tile_pool(name="scr", bufs=2) as scr,
            tc.tile_pool(name="small", bufs=1) as small,
        ):
            # ---- input DMAs.  sync: smalls + full features; gpsimd: gathers
            smalls = consts.tile([128, SMW], fp32, tag="smalls")
            nc.sync.dma_start(smalls, sm_d[:, :])
            wpad = consts.tile([1, KB], bf16, tag="wpad")
            nc.sync.dma_start(wpad, wp_d[:, :])
            fq = []
            for cc in range(CC):
                t = big.tile([128, N], bf16, tag=f"fq{cc}", name=f"fq{cc}")
                nc.sync.dma_start(t, fq_d[c128(cc), :])
                fq.append(t)

            ga = []
            for cc in range(CC):
                t = big.tile([128, GW], bf16, tag=f"ga{cc}", name=f"ga{cc}")
                nc.gpsimd.dma_start(t, ga_d[c128(cc), :])
                ga.append(t)
            gt = []
            for mi in range(MI):
                t = big.tile([128, C], bf16, tag=f"gt{mi}", name=f"gt{mi}")
                nc.gpsimd.dma_start(t, gt_d[c128(mi), :])
                gt.append(t)
            fqa = [g[:, 0:KB] for g in ga]
            fqf = [g[:, KB : KB + KF] for g in ga]
            sfm = [g[:, KB + KF : GW] for g in ga]

            # ---- constants
            ones = consts.tile([128, 128], bf16, tag="ones")
            nc.vector.memset(ones, 1.0)
            ones_f = consts.tile([128, 128], fp32, tag="ones_f")
            nc.vector.memset(ones_f, 1.0)
            # exp bias: {1 -> 0, 0 -> -BIG} from wb-active indicator cols
            biascol = consts.tile([128, MI], fp32, tag="biascol")
            nc.vector.tensor_scalar(
                biascol, smalls[:, 0:MI], BIG, BIG, op0=ALU.mult, op1=ALU.subtract
            )

            # ---- norms of full fq and of the gathered active columns
            rnormB = big.tile([128, N], bf16, tag="rnormB")
            rnA = big.tile([128, KB], bf16, tag="rnA")
            cn = []
            cna = []
            with tc.tile_pool(name="ps_pre", bufs=1, space="PSUM") as ps_pre:
                n2a = ps_pre.tile([128, KB], fp32, tag="n2a", name="n2a")
                for cc in range(CC):
                    sqa_t = scr.tile([128, KB], bf16, tag="sqa", bufs=2, name=f"sqa{cc}")
                    nc.vector.tensor_mul(sqa_t, fqa[cc], fqa[cc])
                    nc.tensor.matmul(
                        n2a, ones, sqa_t, start=(cc == 0), stop=False
                    )
                # pad columns are zero; +1 keeps their rsqrt finite
                nc.tensor.matmul(n2a, ones[0:1, :], wpad, start=False, stop=True)
                tmpa = scr.tile([128, KB], fp32, tag="tmpa", name="tmpa")
                nc.scalar.activation(tmpa, n2a, AF.Ln)
                nc.scalar.activation(rnA, tmpa, AF.Exp, scale=-0.5)

                n2ps = [
                    ps_pre.tile([128, 512], fp32, tag="n2", bufs=2, name=f"n2_{nb}")
                    for nb in range(2)
                ]
                for cc in range(CC):
                    sq = scr.tile([128, N], bf16, tag="sq", bufs=2, name=f"sq{cc}")
                    nc.vector.tensor_mul(sq, fq[cc], fq[cc])
                    for nb in range(2):
                        nc.tensor.matmul(
                            n2ps[nb],
                            ones,
                            sq[:, nbs(nb)],
                            start=(cc == 0),
                            stop=(cc == CC - 1),
                        )
                tmpn = scr.tile([128, N], fp32, tag="tmpn", name="tmpn")
                for nb in range(2):
                    nc.scalar.activation(tmpn[:, nbs(nb)], n2ps[nb], AF.Ln)
                nc.scalar.activation(rnormB, tmpn, AF.Exp, scale=-0.5)

                for cc in range(CC):
                    t = big.tile([128, KB], bf16, tag=f"cna{cc}", name=f"cna{cc}")
                    nc.vector.tensor_mul(t, fqa[cc], rnA)
                    cna.append(t)
                for cc in range(CC):
                    t = big.tile([128, N], bf16, tag=f"cn{cc}", name=f"cn{cc}")
                    nc.vector.tensor_mul(t, fq[cc], rnormB)
                    cn.append(t)

            # ---- per-chunk free-dim reductions for the prototypes (DVE,
            # off the critical path once cn/cna are issued)
            FPc = small.tile([128, CC], fp32, tag="FPc")
            FGc = small.tile([128, CC], fp32, tag="FGc")
            BGc = small.tile([128, CC], fp32, tag="BGc")
            for cc in range(CC):
                nc.vector.reduce_sum(FPc[:, cc : cc + 1], sfm[cc], axis=AX.X)
                nc.vector.reduce_sum(FGc[:, cc : cc + 1], fqf[cc], axis=AX.X)
                nc.vector.reduce_sum(BGc[:, cc : cc + 1], fqa[cc], axis=AX.X)

            # ---- prototype math (tiny, DVE)
            FP1 = small.tile([128, CC], fp32, tag="FP1")
            nc.vector.tensor_scalar_mul(FP1, FPc, smalls[:, MI : MI + 1])
            tmp4 = small.tile([128, CC], fp32, tag="tmp4")
            nc.vector.tensor_scalar_mul(tmp4, FGc, smalls[:, MI + 1 : MI + 2])
            nc.vector.tensor_add(FP1, FP1, tmp4)
            bgp_s = small.tile([128, CC], fp32, tag="bgp_s")
            nc.vector.tensor_scalar_mul(BGc, BGc, smalls[:, MI + 2 : MI + 3])
            sqf = small.tile([128, CC], fp32, tag="sqf")
            nc.vector.tensor_mul(sqf, FP1, FP1)
            rsf = small.tile([128, 1], fp32, tag="rsf")
            nc.vector.reduce_sum(rsf, sqf, axis=AX.X)

            # ---- main phase
            T = [big.tile([128, N], bf16, tag=f"T{mi}", name=f"T{mi}") for mi in range(MI)]
            T2 = [big.tile([128, N], bf16, tag=f"T2{mi}", name=f"T2{mi}") for mi in range(MI)]
            rcolB = big.tile([128, N], bf16, tag="rcolB")
            BP1 = [big.tile([128, N], bf16, tag=f"BP1{cc}", name=f"BP1{cc}") for cc in range(CC)]
            out0 = big.tile([128, N], fp32, tag="out0")
            out1 = small.tile([1, N], fp32, tag="out1", name="out1")
            FP1s = small.tile([128, CC], bf16, tag="FP1s")

            with (
                tc.tile_pool(name="ps_sim", bufs=2, space="PSUM") as ps_sim,
                tc.tile_pool(name="ps_cs", bufs=2, space="PSUM") as ps_cs,
                tc.tile_pool(name="ps_bg", bufs=4, space="PSUM") as ps_bg,
            ):
                # gram + exp + colsum, one 512-col group at a time
                cs = []
                for nb in range(2):
                    cs_t = ps_cs.tile([128, 512], fp32, tag="cs", name=f"cs{nb}")
                    for mi in range(MI):
                        simp = ps_sim.tile(
                            [128, 512], fp32, tag="sim", name=f"sim{nb}_{mi}"
                        )
                        for cc in range(CC):
                            nc.tensor.matmul(
                                simp,
                                cna[cc][:, mi * 128 : (mi + 1) * 128],
                                cn[cc][:, nbs(nb)],
                                start=(cc == 0),
                                stop=(cc == CC - 1),
                            )
                        nc.scalar.activation(
                            T[mi][:, nbs(nb)],
                            simp,
                            AF.Exp,
                            bias=biascol[:, mi : mi + 1],
                            scale=2.0,
                        )
                        nc.tensor.matmul(
                            cs_t,
                            ones,
                            T[mi][:, nbs(nb)],
                            start=(mi == 0),
                            stop=(mi == MI - 1),
                        )
                    cs.append(cs_t)
                    # softmax denominators for this column group (bf16 rcol
                    # is numerically validated end-to-end vs the reference)
                    with nc.allow_low_precision(reason="bf16 softmax denom"):
                        nc.vector.reciprocal(rcolB[:, nbs(nb)], cs_t)
                    for mi in range(MI):
                        nc.vector.tensor_mul(
                            T2[mi][:, nbs(nb)],
                            T[mi][:, nbs(nb)],
                            rcolB[:, nbs(nb)],
                        )

                # bg reconstruction for group 0 (T2 nb=0 ready; nb=1 rescales
                # overlap with these matmuls)
                bg0 = [
                    ps_bg.tile([128, 512], fp32, tag="bg", name=f"bg0_{cc}")
                    for cc in range(CC)
                ]
                for mi in range(MI):
                    for cc in range(CC):
                        nc.tensor.matmul(
                            bg0[cc],
                            gt[mi][:, c128(cc)],
                            T2[mi][:, nbs(0)],
                            start=(mi == 0),
                            stop=(mi == MI - 1),
                        )

                # fg path: nfp2 = ||FP1||^2, f10 = 10/sqrt(nfp2), FP1s, dfg
                nfp_ps = ps_cs.tile([1, 1], fp32, tag="cs", name="nfp")
                nc.tensor.matmul(nfp_ps, ones_f[:, 0:1], rsf, start=True, stop=True)
                nfp_sb = small.tile([1, 1], fp32, tag="nfp_sb")
                nc.vector.tensor_copy(nfp_sb, nfp_ps)
                f10_ps = ps_cs.tile([128, 1], fp32, tag="cs", name="f10p")
                nc.tensor.matmul(f10_ps, ones_f[0:1, :], nfp_sb, start=True, stop=True)
                f10r = small.tile([128, 1], fp32, tag="f10r")
                nc.vector.reciprocal(f10r, f10_ps)
                f10B = small.tile([128, 1], fp32, tag="f10B")
                nc.scalar.activation(f10B, f10r, AF.Sqrt, scale=100.0)
                nc.vector.tensor_scalar_mul(FP1s, FP1, f10B)
                dfg = []
                for nb in range(2):
                    d_t = ps_cs.tile([1, 512], fp32, tag="cs", name=f"dfg{nb}")
                    for cc in range(CC):
                        nc.tensor.matmul(
                            d_t,
                            FP1s[:, cc : cc + 1],
                            cn[cc][:, nbs(nb)],
                            start=(cc == 0),
                            stop=(cc == CC - 1),
                        )
                    dfg.append(d_t)
                for nb in range(2):
                    nc.scalar.copy(out1[:, nbs(nb)], dfg[nb])
                nc.sync.dma_start(out_d[1:2, :], out1)

                # bg reconstruction for group 1
                bg1 = [
                    ps_bg.tile([128, 512], fp32, tag="bg", name=f"bg1_{cc}")
                    for cc in range(CC)
                ]
                for mi in range(MI):
                    for cc in range(CC):
                        nc.tensor.matmul(
                            bg1[cc],
                            gt[mi][:, c128(cc)],
                            T2[mi][:, nbs(1)],
                            start=(mi == 0),
                            stop=(mi == MI - 1),
                        )

                # BP1 = recon + (3/7) bg_proto  (psum -> sbuf, bias add)
                for nb, bg in ((0, bg0), (1, bg1)):
                    for cc in range(CC):
                        if cc < 2:
                            nc.scalar.activation(
                                BP1[cc][:, nbs(nb)],
                                bg[cc],
                                AF.Identity,
                                bias=BGc[:, cc : cc + 1],
                            )
                        else:
                            nc.vector.tensor_scalar_add(
                                BP1[cc][:, nbs(nb)], bg[cc], BGc[:, cc : cc + 1]
                            )

                # final bg similarity: usum = cn.BP1, qsum = |BP1|^2,
                # out0 = usum * 10/sqrt(qsum)
                us = [
                    ps_bg.tile([128, 512], fp32, tag="bg", name=f"us{nb}")
                    for nb in range(2)
                ]
                qs = [
                    ps_bg.tile([128, 512], fp32, tag="bg", name=f"qs{nb}")
                    for nb in range(2)
                ]
                for cc in range(CC):
                    p_t = scr.tile([128, N], bf16, tag="p", bufs=2, name=f"p{cc}")
                    nc.vector.tensor_mul(p_t, cn[cc], BP1[cc])
                    q_t = scr.tile([128, N], bf16, tag="q", bufs=2, name=f"q{cc}")
                    nc.vector.tensor_mul(q_t, BP1[cc], BP1[cc])
                    for nb in range(2):
                        nc.tensor.matmul(
                            us[nb],
                            ones,
                            p_t[:, nbs(nb)],
                            start=(cc == 0),
                            stop=(cc == CC - 1),
                        )
                        nc.tensor.matmul(
                            qs[nb],
                            ones,
                            q_t[:, nbs(nb)],
                            start=(cc == 0),
                            stop=(cc == CC - 1),
                        )
                rq = scr.tile([128, N], fp32, tag="rq", name="rq")
                r1 = scr.tile([128, N], fp32, tag="r1", name="r1")
                for nb in range(2):
                    nc.vector.reciprocal(rq[:, nbs(nb)], qs[nb])
                    nc.scalar.activation(
                        r1[:, nbs(nb)], rq[:, nbs(nb)], AF.Sqrt, scale=100.0
                    )
                    nc.vector.tensor_mul(out0[:, nbs(nb)], us[nb], r1[:, nbs(nb)])
                nc.sync.dma_start(out_d[0:1, :], out0[0:1, :])

    if split_waits:
        _split_multi_waits(nc)
    return nc


def _get_nc(KB, KF, KM):
    key = (KB, KF, KM)
    if key not in _cache:
        _cache[key] = _build_nc(KB, KF, KM)
    return _cache[key]


# --------------------------------------------------------------------------
# host prep: gathers + scalars
# --------------------------------------------------------------------------
def _round_up(x, m):
    return ((x + m - 1) // m) * m


def _make_in_maps(feature_q, support_feat, support_mask):
    wf, wb = _host_select_weights(feature_q, support_feat, support_mask)
    fqr = feature_q.reshape(B, C, N).astype(ml_dtypes.bfloat16)
    sfr = support_feat.reshape(B, C, N).astype(ml_dtypes.bfloat16)
    mfr = support_mask.reshape(B, N) == 1

    nb_ = wb.sum(1).astype(int)
    nf_ = wf.sum(1).astype(int)
    nm_ = mfr.sum(1).astype(int)
    KB = max(KB0, _round_up(nb_.max() + 1, 128))
    KF = max(KF0, _round_up(nf_.max(), 64))
    KM = max(KM0, _round_up(max(nm_.max(), 1), 64))
    MI = KB // 128

    in_maps = []
    for b in range(B):
        ib = np.where(wb[b] > 0)[0]
        iff = np.where(wf[b] > 0)[0]
        im = np.where(mfr[b])[0]
        ga = np.zeros((C, KB + KF + KM), ml_dtypes.bfloat16)
        ga[:, : len(ib)] = fqr[b][:, ib]
        ga[:, KB : KB + len(iff)] = fqr[b][:, iff]
        ga[:, KB + KF : KB + KF + len(im)] = sfr[b][:, im]
        gt = np.zeros((KB, C), ml_dtypes.bfloat16)
        gt[: len(ib)] = fqr[b][:, ib].T
        wba = np.zeros(KB, np.float32)
        wba[: len(ib)] = 1.0
        sm = np.zeros((128, MI + 5), np.float32)
        sm[:, 0:MI] = wba.reshape(MI, 128).T
        sm[:, MI] = 1.0 / (nm_[b] + 1e-5)
        sm[:, MI + 1] = 1.0 / max(nf_[b], 1)
        sm[:, MI + 2] = (3.0 / 7.0) / max(nb_[b], 1)
        wp = (1.0 - wba).astype(ml_dtypes.bfloat16)[None, :]
        in_maps.append(
            {"fq": fqr[b], "ga": ga, "gt": gt, "sm": sm, "wp": wp}
        )
    return in_maps, (KB, KF, KM)


def run_sharded(feature_q, support_feat, support_mask, **kwargs):
    """Run on all 8 cores; returns (output [B,2,H,W], BassKernelResults)."""
    from concourse.bass_utils import run_bass_kernel_spmd

    in_maps, caps = _make_in_maps(
        np.asarray(feature_q), np.asarray(support_feat), np.asarray(support_mask)
    )
    nc = _get_nc(*caps)
    res = run_bass_kernel_spmd(nc, in_maps, core_ids=list(range(B)), **kwargs)
    out = np.stack([res.results[b]["out"] for b in range(B)])
    return out.reshape(B, 2, H, W).astype(np.float32), res


def kernel(feature_q, support_feat, support_mask):
    out, _ = run_sharded(
        np.asarray(feature_q), np.asarray(support_feat), np.asarray(support_mask)
    )
    return out


# revision 9
# speedup vs baseline: 2.5955x; 1.1393x over previous
"""Trainium2 Bass kernel for DFBNet SSP (sparse_attention).

Data-parallel over batch: 8 samples -> 8 NeuronCores, one sample per core.

Sparse formulation: the reference's bg softmax masks to the wb-active columns
(|wb| ~ 270-320 of N=1024), so the [N,N] gram is really [KB,N] with KB the
padded active count.  The host computes the discrete {0,1} selection vectors
(exact fp64 replica of the reference pred chain incl. top-k fallback), turns
them into index gathers of the bf16-rounded inputs, and ships:

  fq    [C, N]   bf16   full features (sim rhs, norms)
  ga    [C, GW]  bf16   [fqa | fqf | sfm] gathered cols, zero-padded
  gt    [KB, C]  bf16   fqa transposed (recon stationary)
  sm    [128, 3+MI] f32 wb-active indicator (chunk layout) + 1/count scalars
  wp    [1, KB]  bf16   pad indicator row (fixes norms of zero pad cols)

Device (per core) computes everything continuous:
  norms + cn = fq/||fq||, cna; sim = cna^T cn [KB,N]; T = wb*exp(2 sim)
  (additive -BIG mask in the Exp bias); colsum via ones-matmul; rcol = 1/cs;
  T' = T*rcol; recon = fqa @ T' (= bg_local); BP1 = recon + (3/7)bg_proto;
  out0 = 10*cos(fq,BP1) via cn-folded dots; FP1 = FP + fg_proto (cosine
  scale-invariance drops the reference's 0.5/0.5 and 0.3/0.7 blend scales);
  out1 = (FP1*10/||FP1||)^T cn.
"""

import numpy as np
import ml_dtypes

B, C, H, W = 8, 512, 32, 32
N = H * W
CC = C // 128  # 4 channel chunks
FG_THRES, BG_THRES, TOPK = 0.7, 0.6, 12
BIG = 60000.0
LN10 = 2.302585092994046

# default gather capacities (multiples: KB of 128; KF/KM of 64)
KB0, KF0, KM0 = 384, 256, 640

_cache = {}


# --------------------------------------------------------------------------
# host: selection weights (exact reference semantics, float64)
# --------------------------------------------------------------------------
def _host_select_weights(feature_q, support_feat, support_mask):
    fq = feature_q.astype(np.float64).reshape(B, C, N)
    sf = support_feat.astype(np.float64).reshape(B, C, N)
    mf = (support_mask.reshape(B, N) == 1).astype(np.float64)
    mb = 1.0 - mf
    FP = (sf * mf[:, None]).sum(-1) / (mf.sum(-1)[:, None] + 1e-5)
    BP = (sf * mb[:, None]).sum(-1) / (mb.sum(-1)[:, None] + 1e-5)

    def cos(a, b):  # a [B,C,N], b [B,C]
        dot = (a * b[:, :, None]).sum(1)
        na = np.sqrt((a * a).sum(1))
        nb = np.sqrt((b * b).sum(1))[:, None]
        return dot / np.maximum(na * nb, 1e-8)

    sfg = cos(fq, FP) * 10.0
    sbg = cos(fq, BP) * 10.0
    m = np.maximum(sfg, sbg)
    efg = np.exp(sfg - m)
    ebg = np.exp(sbg - m)
    pfg = efg / (efg + ebg)
    pbg = ebg / (efg + ebg)

    def select(pred, thres):
        w = np.zeros((B, N), np.float32)
        for b in range(B):
            row = pred[b] > thres
            if row.sum() > 0:
                w[b] = row
            else:
                # jax.lax.top_k tie-break: lower index wins -> stable argsort
                idx = np.argsort(-pred[b], kind="stable")[:TOPK]
                w[b, idx] = 1.0
        return w

    return select(pfg, FG_THRES), select(pbg, BG_THRES)


# --------------------------------------------------------------------------
# build-environment workarounds (this walrus build's sync-wait limits)
# --------------------------------------------------------------------------
def _make_tile_context_cls():
    import concourse.tile as tile
    from concourse.vector_clock import ScopedClock, VectorClock

    class PatchedTileContext(tile.TileContext):
        """This walrus build rejects CTRL/Drain instructions carrying more
        than one sem wait.  Put the tail-drain's global-clock waits on
        single-wait NOPs (same engine, program order) instead."""

        def _drain_and_barrier(self, tick_clock, wait_clock):
            gc = tick_clock.global_clock
            n = len(gc)
            for proc in range(n):
                t = gc[proc]
                if t > 0:
                    vec = [0] * n
                    vec[proc] = t
                    nop = self.nc.sync.nop(nofuse=True)
                    wait_clock.add_sem_waits(
                        nop.ins, ScopedClock({None: VectorClock(vec)})
                    )
            self.nc.sync.drain()
            self.nc.all_engine_barrier()
            assert self.sems is not None
            popped = self.nc._tile_sem_poison_stack.pop()
            assert popped is self._sem_poison
            self.nc.clear_and_free_semaphores(list(self.sems.allocated().values()))
            self.nc.all_engine_barrier()

    return PatchedTileContext


def _split_multi_waits(nc):
    """This walrus build allows at most one sync-wait command per
    instruction.  Move extra waits onto same-engine NOPs inserted just
    before the instruction (waits are AND conditions; order-safe)."""
    import concourse.mybir as mybir

    n_split = 0
    for f in nc.m.functions:
        for bb in f.blocks:
            il = bb.instructions
            i = 0
            while i < len(il):
                inst = il[i]
                si = inst.sync_info
                if si is not None and si.on_wait and len(si.on_wait) > 1:
                    waits = list(si.on_wait)
                    for j, w in enumerate(waits[:-1]):
                        nop = mybir.InstNoOp(
                            name=f"{inst.name}-wsplit{j}",
                            ins=[],
                            outs=[],
                            engine=inst.engine,
                            sync_info=mybir.SyncInfo(on_wait=[w], on_update=[]),
                        )
                        il.insert(i, nop)
                        i += 1
                        n_split += 1
                    inst.sync_info = mybir.SyncInfo(
                        on_wait=[waits[-1]], on_update=si.on_update
                    )
                i += 1
    return n_split


# --------------------------------------------------------------------------
# device program
# --------------------------------------------------------------------------
def _build_nc(KB, KF, KM, split_waits=True):
    import concourse.bass as bass
    import concourse.mybir as mybir

    fp32 = mybir.dt.float32
    bf16 = mybir.dt.bfloat16
    AF = mybir.ActivationFunctionType
    ALU = mybir.AluOpType
    AX = mybir.AxisListType

    MI = KB // 128  # active-row chunks
    GW = KB + KF + KM  # packed gather width per channel chunk
    SMW = MI + 5

    PatchedTileContext = _make_tile_context_cls()

    nc = bass.Bass("TRN2", target_bir_lowering=False)
    fq_d = nc.declare_dram_parameter("fq", [C, N], bf16, isOutput=False)
    ga_d = nc.declare_dram_parameter("ga", [C, GW], bf16, isOutput=False)
    gt_d = nc.declare_dram_parameter("gt", [KB, C], bf16, isOutput=False)
    sm_d = nc.declare_dram_parameter("sm", [128, SMW], fp32, isOutput=False)
    wp_d = nc.declare_dram_parameter("wp", [1, KB], bf16, isOutput=False)
    out_d = nc.declare_dram_parameter("out", [2, N], fp32, isOutput=True)

    def nbs(nb):
        return slice(nb * 512, (nb + 1) * 512)

    def c128(cc):
        return slice(cc * 128, (cc + 1) * 128)

    with PatchedTileContext(nc) as tc:
        with (
            tc.tile_pool(name="consts", bufs=1) as consts,
            tc.tile_pool(name="big", bufs=1) as big,
            tc.tile_pool(name="scr", bufs=2) as scr,
            tc.tile_pool(name="small", bufs=1) as small,
        ):
            # ---- input DMAs spread across issue queues: fq on sync,
            # gathers on gpsimd, smalls + transposed gather on scalar
            smalls = consts.tile([128, SMW], fp32, tag="smalls")
            nc.scalar.dma_start(smalls, sm_d[:, :])
            wpad = consts.tile([1, KB], bf16, tag="wpad")
            nc.scalar.dma_start(wpad, wp_d[:, :])
            fq = []
            for cc in range(CC):
                t = big.tile([128, N], bf16, tag=f"fq{cc}", name=f"fq{cc}")
                nc.sync.dma_start(t, fq_d[c128(cc), :])
                fq.append(t)
            ga = []
            for cc in range(CC):
                t = big.tile([128, GW], bf16, tag=f"ga{cc}", name=f"ga{cc}")
                nc.gpsimd.dma_start(t, ga_d[c128(cc), :])
                ga.append(t)
            gt = []
            for mi in range(MI):
                t = big.tile([128, C], bf16, tag=f"gt{mi}", name=f"gt{mi}")
                nc.scalar.dma_start(t, gt_d[c128(mi), :])
                gt.append(t)
            fqa = [g[:, 0:KB] for g in ga]
            fgm = [g[:, KB:GW] for g in ga]  # pre-scaled [fqf*rcf | sfm*rcm]

            # ---- constants
            ones = consts.tile([128, 128], bf16, tag="ones")
            nc.vector.memset(ones, 1.0)
            ones_f = consts.tile([128, 128], fp32, tag="ones_f")
            nc.vector.memset(ones_f, 1.0)
            ln10B = consts.tile([128, 1], fp32, tag="ln10B")
            nc.vector.memset(ln10B, LN10)
            # exp bias: {1 -> 0, 0 -> -BIG} from wb-active indicator cols
            biascol = consts.tile([128, MI], fp32, tag="biascol")
            nc.vector.tensor_scalar(
                biascol, smalls[:, 0:MI], BIG, BIG, op0=ALU.mult, op1=ALU.subtract
            )

            # ---- norms of the gathered active columns, then of full fq
            rnormB = big.tile([128, N], bf16, tag="rnormB")
            rnA = big.tile([128, KB], bf16, tag="rnA")
            cn = []
            cna = []
            with tc.tile_pool(name="ps_pre", bufs=1, space="PSUM") as ps_pre:
                n2a = ps_pre.tile([128, KB], fp32, tag="n2a", name="n2a")
                for cc in range(CC):
                    sqa_t = scr.tile([128, KB], bf16, tag="sqa", bufs=2, name=f"sqa{cc}")
                    nc.vector.tensor_mul(sqa_t, fqa[cc], fqa[cc])
                    nc.tensor.matmul(n2a, ones, sqa_t, start=(cc == 0), stop=False)
                # pad columns are zero; +1 keeps their rsqrt finite
                nc.tensor.matmul(n2a, ones[0:1, :], wpad, start=False, stop=True)
                tmpa = scr.tile([128, KB], fp32, tag="tmpa", name="tmpa")
                nc.scalar.activation(tmpa, n2a, AF.Ln)
                nc.scalar.activation(rnA, tmpa, AF.Exp, scale=-0.5)

                n2ps = [
                    ps_pre.tile([128, 512], fp32, tag="n2", bufs=2, name=f"n2_{nb}")
                    for nb in range(2)
                ]
                for cc in range(CC):
                    sq = scr.tile([128, N], bf16, tag="sq", bufs=2, name=f"sq{cc}")
                    nc.vector.tensor_mul(sq, fq[cc], fq[cc])
                    for nb in range(2):
                        nc.tensor.matmul(
                            n2ps[nb],
                            ones,
                            sq[:, nbs(nb)],
                            start=(cc == 0),
                            stop=(cc == CC - 1),
                        )
                tmpn = scr.tile([128, N], fp32, tag="tmpn", name="tmpn")
                for nb in range(2):
                    nc.scalar.activation(tmpn[:, nbs(nb)], n2ps[nb], AF.Ln)
                nc.scalar.activation(rnormB, tmpn, AF.Exp, scale=-0.5)

                for cc in range(CC):
                    t = big.tile([128, KB], bf16, tag=f"cna{cc}", name=f"cna{cc}")
                    nc.vector.tensor_mul(t, fqa[cc], rnA)
                    cna.append(t)
                for cc in range(CC):
                    t = big.tile([128, N], bf16, tag=f"cn{cc}", name=f"cn{cc}")
                    nc.vector.tensor_mul(t, fq[cc], rnormB)
                    cn.append(t)

            # ---- prototype reductions (DVE, off the critical path).  The
            # fg/mask gathers are host-pre-scaled by 1/count, so FP1 = FP +
            # fg_proto is a single segment sum per channel chunk.
            FP1 = small.tile([128, CC], fp32, tag="FP1")
            BGc = small.tile([128, CC], fp32, tag="BGc")
            for cc in range(CC):
                nc.vector.reduce_sum(FP1[:, cc : cc + 1], fgm[cc], axis=AX.X)
                nc.vector.reduce_sum(BGc[:, cc : cc + 1], fqa[cc], axis=AX.X)
            nc.vector.tensor_scalar_mul(BGc, BGc, smalls[:, MI : MI + 1])
            sqf = small.tile([128, CC], fp32, tag="sqf")
            nc.vector.tensor_mul(sqf, FP1, FP1)
            rsf = small.tile([128, 1], fp32, tag="rsf")
            nc.vector.reduce_sum(rsf, sqf, axis=AX.X)

            # ---- main phase
            T = [big.tile([128, N], bf16, tag=f"T{mi}", name=f"T{mi}") for mi in range(MI)]
            T2 = [big.tile([128, N], bf16, tag=f"T2{mi}", name=f"T2{mi}") for mi in range(MI)]
            rcolB = big.tile([128, N], bf16, tag="rcolB")
            BP1 = [big.tile([128, N], bf16, tag=f"BP1{cc}", name=f"BP1{cc}") for cc in range(CC)]
            out0 = big.tile([128, N], fp32, tag="out0")
            out1 = small.tile([1, N], fp32, tag="out1", name="out1")
            FP1s = small.tile([128, CC], bf16, tag="FP1s")

            with (
                tc.tile_pool(name="ps_sim", bufs=2, space="PSUM") as ps_sim,
                tc.tile_pool(name="ps_cs", bufs=2, space="PSUM") as ps_cs,
                tc.tile_pool(name="ps_bg", bufs=4, space="PSUM") as ps_bg,
            ):
                # gram + exp + colsum, one 512-col group at a time
                tcs = scr.tile([128, N], fp32, tag="tcs", name="tcs")
                for nb in range(2):
                    cs_t = ps_cs.tile([128, 512], fp32, tag="cs", name=f"cs{nb}")
                    for mi in range(MI):
                        simp = ps_sim.tile(
                            [128, 512], fp32, tag="sim", name=f"sim{nb}_{mi}"
                        )
                        for cc in range(CC):
                            nc.tensor.matmul(
                                simp,
                                cna[cc][:, mi * 128 : (mi + 1) * 128],
                                cn[cc][:, nbs(nb)],
                                start=(cc == 0),
                                stop=(cc == CC - 1),
                            )
                        nc.scalar.activation(
                            T[mi][:, nbs(nb)],
                            simp,
                            AF.Exp,
                            bias=biascol[:, mi : mi + 1],
                            scale=2.0,
                        )
                        nc.tensor.matmul(
                            cs_t,
                            ones,
                            T[mi][:, nbs(nb)],
                            start=(mi == 0),
                            stop=(mi == MI - 1),
                        )
                    # softmax denominators: rcol = exp(-ln(colsum))
                    nc.scalar.activation(tcs[:, nbs(nb)], cs_t, AF.Ln)
                    nc.scalar.activation(
                        rcolB[:, nbs(nb)], tcs[:, nbs(nb)], AF.Exp, scale=-1.0
                    )

                # fg norm scale 10/||FP1|| replicated across partitions
                nfp_ps = ps_cs.tile([1, 1], fp32, tag="cs", name="nfp")
                nc.tensor.matmul(nfp_ps, ones_f[:, 0:1], rsf, start=True, stop=True)
                nfp_sb = small.tile([1, 1], fp32, tag="nfp_sb")
                nc.vector.tensor_copy(nfp_sb, nfp_ps)
                f10_ps = ps_cs.tile([128, 1], fp32, tag="cs", name="f10p")
                nc.tensor.matmul(f10_ps, ones_f[0:1, :], nfp_sb, start=True, stop=True)
                f10a = small.tile([128, 1], fp32, tag="f10a")
                nc.scalar.activation(f10a, f10_ps, AF.Ln)
                f10B = small.tile([128, 1], fp32, tag="f10B")
                nc.scalar.activation(f10B, f10a, AF.Exp, scale=-0.5, bias=ln10B)

                # softmax renormalization folded into T (frees the recon psum
                # accumulation to produce BP1 - bg_proto directly)
                for mi in range(MI):
                    nc.vector.tensor_mul(T2[mi], T[mi], rcolB)
                nc.vector.tensor_scalar_mul(FP1s, FP1, f10B)

                # bg reconstruction for group 0 (T2 nb=1 half overlaps)
                bg0 = [
                    ps_bg.tile([128, 512], fp32, tag="bg", name=f"bg0_{cc}")
                    for cc in range(CC)
                ]
                for mi in range(MI):
                    for cc in range(CC):
                        nc.tensor.matmul(
                            bg0[cc],
                            gt[mi][:, c128(cc)],
                            T2[mi][:, nbs(0)],
                            start=(mi == 0),
                            stop=(mi == MI - 1),
                        )

                # fg similarity rows (cn-folded: out1 = FP1s^T cn)
                dfg = []
                for nb in range(2):
                    d_t = ps_cs.tile([1, 512], fp32, tag="cs", name=f"dfg{nb}")
                    for cc in range(CC):
                        nc.tensor.matmul(
                            d_t,
                            FP1s[:, cc : cc + 1],
                            cn[cc][:, nbs(nb)],
                            start=(cc == 0),
                            stop=(cc == CC - 1),
                        )
                    dfg.append(d_t)

                # bg reconstruction for group 1
                bg1 = [
                    ps_bg.tile([128, 512], fp32, tag="bg", name=f"bg1_{cc}")
                    for cc in range(CC)
                ]
                for mi in range(MI):
                    for cc in range(CC):
                        nc.tensor.matmul(
                            bg1[cc],
                            gt[mi][:, c128(cc)],
                            T2[mi][:, nbs(1)],
                            start=(mi == 0),
                            stop=(mi == MI - 1),
                        )

                # BP1 = recon + (3/7) bg_proto  (psum -> sbuf, bias add),
                # split scalar/DVE
                for nb, bg in ((0, bg0), (1, bg1)):
                    for cc in range(CC):
                        if cc < 2:
                            nc.scalar.activation(
                                BP1[cc][:, nbs(nb)],
                                bg[cc],
                                AF.Identity,
                                bias=BGc[:, cc : cc + 1],
                            )
                        else:
                            nc.vector.tensor_scalar_add(
                                BP1[cc][:, nbs(nb)], bg[cc], BGc[:, cc : cc + 1]
                            )
                    if nb == 0:
                        for k in range(2):
                            nc.scalar.copy(out1[:, nbs(k)], dfg[k])
                        nc.sync.dma_start(out_d[1:2, :], out1)

                # final bg similarity: usum = cn.BP1, qsum = |BP1|^2,
                # out0 = usum * exp(ln10 - 0.5 ln(qsum))
                us = [
                    ps_bg.tile([128, 512], fp32, tag="bg", name=f"us{nb}")
                    for nb in range(2)
                ]
                qs = [
                    ps_bg.tile([128, 512], fp32, tag="bg", name=f"qs{nb}")
                    for nb in range(2)
                ]
                for cc in range(CC):
                    p_t = scr.tile([128, N], bf16, tag="p", bufs=2, name=f"p{cc}")
                    nc.vector.tensor_mul(p_t, cn[cc], BP1[cc])
                    q_t = scr.tile([128, N], bf16, tag="q", bufs=2, name=f"q{cc}")
                    nc.vector.tensor_mul(q_t, BP1[cc], BP1[cc])
                    for nb in range(2):
                        nc.tensor.matmul(
                            us[nb],
                            ones,
                            p_t[:, nbs(nb)],
                            start=(cc == 0),
                            stop=(cc == CC - 1),
                        )
                        nc.tensor.matmul(
                            qs[nb],
                            ones,
                            q_t[:, nbs(nb)],
                            start=(cc == 0),
                            stop=(cc == CC - 1),
                        )
                trq = scr.tile([128, N], fp32, tag="trq", name="trq")
                r1 = scr.tile([128, N], fp32, tag="r1", name="r1")
                for nb in range(2):
                    nc.scalar.activation(trq[:, nbs(nb)], qs[nb], AF.Ln)
                    nc.scalar.activation(
                        r1[:, nbs(nb)], trq[:, nbs(nb)], AF.Exp,
                        scale=-0.5, bias=ln10B,
                    )
                    nc.vector.tensor_mul(out0[:, nbs(nb)], us[nb], r1[:, nbs(nb)])
                nc.sync.dma_start(out_d[0:1, :], out0[0:1, :])

    if split_waits:
        _split_multi_waits(nc)
    return nc


def _get_nc(KB, KF, KM):
    key = (KB, KF, KM)
    if key not in _cache:
        _cache[key] = _build_nc(KB, KF, KM)
    return _cache[key]


# --------------------------------------------------------------------------
# host prep: gathers + scalars
# --------------------------------------------------------------------------
def _round_up(x, m):
    return ((x + m - 1) // m) * m


def _make_in_maps(feature_q, support_feat, support_mask):
    wf, wb = _host_select_weights(feature_q, support_feat, support_mask)
    fqr = feature_q.reshape(B, C, N).astype(ml_dtypes.bfloat16)
    sfr = support_feat.reshape(B, C, N).astype(ml_dtypes.bfloat16)
    mfr = support_mask.reshape(B, N) == 1

    nb_ = wb.sum(1).astype(int)
    nf_ = wf.sum(1).astype(int)
    nm_ = mfr.sum(1).astype(int)
    KB = max(KB0, _round_up(nb_.max() + 1, 128))
    KF = max(KF0, _round_up(nf_.max(), 64))
    KM = max(KM0, _round_up(max(nm_.max(), 1), 64))
    MI = KB // 128

    in_maps = []
    for b in range(B):
        ib = np.where(wb[b] > 0)[0]
        iff = np.where(wf[b] > 0)[0]
        im = np.where(mfr[b])[0]
        rcf = np.float32(1.0 / max(nf_[b], 1))
        rcm = np.float32(1.0 / (nm_[b] + 1e-5))
        ga = np.zeros((C, KB + KF + KM), ml_dtypes.bfloat16)
        ga[:, : len(ib)] = fqr[b][:, ib]
        ga[:, KB : KB + len(iff)] = (
            fqr[b][:, iff].astype(np.float32) * rcf
        ).astype(ml_dtypes.bfloat16)
        ga[:, KB + KF : KB + KF + len(im)] = (
            sfr[b][:, im].astype(np.float32) * rcm
        ).astype(ml_dtypes.bfloat16)
        gt = np.zeros((KB, C), ml_dtypes.bfloat16)
        gt[: len(ib)] = fqr[b][:, ib].T
        wba = np.zeros(KB, np.float32)
        wba[: len(ib)] = 1.0
        sm = np.zeros((128, MI + 5), np.float32)
        sm[:, 0:MI] = wba.reshape(MI, 128).T
        sm[:, MI] = (3.0 / 7.0) / max(nb_[b], 1)
        wp = (1.0 - wba).astype(ml_dtypes.bfloat16)[None, :]
        in_maps.append(
            {"fq": fqr[b], "ga": ga, "gt": gt, "sm": sm, "wp": wp}
        )
    return in_maps, (KB, KF, KM)


def run_sharded(feature_q, support_feat, support_mask, **kwargs):
    """Run on all 8 cores; returns (output [B,2,H,W], BassKernelResults)."""
    from concourse.bass_utils import run_bass_kernel_spmd

    in_maps, caps = _make_in_maps(
        np.asarray(feature_q), np.asarray(support_feat), np.asarray(support_mask)
    )
    nc = _get_nc(*caps)
    res = run_bass_kernel_spmd(nc, in_maps, core_ids=list(range(B)), **kwargs)
    out = np.stack([res.results[b]["out"] for b in range(B)])
    return out.reshape(B, 2, H, W).astype(np.float32), res


def kernel(feature_q, support_feat, support_mask):
    out, _ = run_sharded(
        np.asarray(feature_q), np.asarray(support_feat), np.asarray(support_mask)
    )
    return out


# revision 13
# speedup vs baseline: 2.7181x; 1.0472x over previous
"""Trainium2 Bass kernel for DFBNet SSP (sparse_attention).

Data-parallel over batch: 8 samples -> 8 NeuronCores, one sample per core.

Sparse formulation: the reference's bg softmax masks to the wb-active columns
(|wb| ~ 270-320 of N=1024), so the [N,N] gram is really [KB,N] with KB the
padded active count.  The host computes the discrete {0,1} selection vectors
(exact fp64 replica of the reference pred chain incl. top-k fallback), turns
them into index gathers of the bf16-rounded inputs, and ships:

  fq    [C, N]   bf16   full features (sim rhs, norms)
  ga    [C, GW]  bf16   [fqa | fqf | sfm] gathered cols, zero-padded
  gt    [KB, C]  bf16   fqa transposed (recon stationary)
  sm    [128, 3+MI] f32 wb-active indicator (chunk layout) + 1/count scalars
  wp    [1, KB]  bf16   pad indicator row (fixes norms of zero pad cols)

Device (per core) computes everything continuous:
  norms + cn = fq/||fq||, cna; sim = cna^T cn [KB,N]; T = wb*exp(2 sim)
  (additive -BIG mask in the Exp bias); colsum via ones-matmul; rcol = 1/cs;
  T' = T*rcol; recon = fqa @ T' (= bg_local); BP1 = recon + (3/7)bg_proto;
  out0 = 10*cos(fq,BP1) via cn-folded dots; FP1 = FP + fg_proto (cosine
  scale-invariance drops the reference's 0.5/0.5 and 0.3/0.7 blend scales);
  out1 = (FP1*10/||FP1||)^T cn.
"""

import numpy as np
import ml_dtypes

B, C, H, W = 8, 512, 32, 32
N = H * W
CC = C // 128  # 4 channel chunks
FG_THRES, BG_THRES, TOPK = 0.7, 0.6, 12
BIG = 60000.0
LN10 = 2.302585092994046
LN2 = 0.6931471805599453

# default gather capacities (multiples: KB of 128; KF/KM of 64)
KB0, KF0, KM0 = 384, 256, 640

_cache = {}


# --------------------------------------------------------------------------
# host: selection weights (exact reference semantics, float64)
# --------------------------------------------------------------------------
def _host_select_weights(feature_q, support_feat, support_mask):
    fq = feature_q.astype(np.float64).reshape(B, C, N)
    sf = support_feat.astype(np.float64).reshape(B, C, N)
    mf = (support_mask.reshape(B, N) == 1).astype(np.float64)
    mb = 1.0 - mf
    FP = (sf * mf[:, None]).sum(-1) / (mf.sum(-1)[:, None] + 1e-5)
    BP = (sf * mb[:, None]).sum(-1) / (mb.sum(-1)[:, None] + 1e-5)

    def cos(a, b):  # a [B,C,N], b [B,C]
        dot = (a * b[:, :, None]).sum(1)
        na = np.sqrt((a * a).sum(1))
        nb = np.sqrt((b * b).sum(1))[:, None]
        return dot / np.maximum(na * nb, 1e-8)

    sfg = cos(fq, FP) * 10.0
    sbg = cos(fq, BP) * 10.0
    m = np.maximum(sfg, sbg)
    efg = np.exp(sfg - m)
    ebg = np.exp(sbg - m)
    pfg = efg / (efg + ebg)
    pbg = ebg / (efg + ebg)

    def select(pred, thres):
        w = np.zeros((B, N), np.float32)
        for b in range(B):
            row = pred[b] > thres
            if row.sum() > 0:
                w[b] = row
            else:
                # jax.lax.top_k tie-break: lower index wins -> stable argsort
                idx = np.argsort(-pred[b], kind="stable")[:TOPK]
                w[b, idx] = 1.0
        return w

    return select(pfg, FG_THRES), select(pbg, BG_THRES)


# --------------------------------------------------------------------------
# build-environment workarounds (this walrus build's sync-wait limits)
# --------------------------------------------------------------------------
def _make_tile_context_cls():
    import concourse.tile as tile
    from concourse.vector_clock import ScopedClock, VectorClock

    class PatchedTileContext(tile.TileContext):
        """This walrus build rejects CTRL/Drain instructions carrying more
        than one sem wait.  Put the tail-drain's global-clock waits on
        single-wait NOPs (same engine, program order) instead."""

        def _drain_and_barrier(self, tick_clock, wait_clock):
            gc = tick_clock.global_clock
            n = len(gc)
            for proc in range(n):
                t = gc[proc]
                if t > 0:
                    vec = [0] * n
                    vec[proc] = t
                    nop = self.nc.sync.nop(nofuse=True)
                    wait_clock.add_sem_waits(
                        nop.ins, ScopedClock({None: VectorClock(vec)})
                    )
            self.nc.sync.drain()
            self.nc.all_engine_barrier()
            assert self.sems is not None
            popped = self.nc._tile_sem_poison_stack.pop()
            assert popped is self._sem_poison
            self.nc.clear_and_free_semaphores(list(self.sems.allocated().values()))
            self.nc.all_engine_barrier()

    return PatchedTileContext


def _split_multi_waits(nc):
    """This walrus build allows at most one sync-wait command per
    instruction.  Move extra waits onto same-engine NOPs inserted just
    before the instruction (waits are AND conditions; order-safe)."""
    import concourse.mybir as mybir

    n_split = 0
    for f in nc.m.functions:
        for bb in f.blocks:
            il = bb.instructions
            i = 0
            while i < len(il):
                inst = il[i]
                si = inst.sync_info
                if si is not None and si.on_wait and len(si.on_wait) > 1:
                    waits = list(si.on_wait)
                    for j, w in enumerate(waits[:-1]):
                        nop = mybir.InstNoOp(
                            name=f"{inst.name}-wsplit{j}",
                            ins=[],
                            outs=[],
                            engine=inst.engine,
                            sync_info=mybir.SyncInfo(on_wait=[w], on_update=[]),
                        )
                        il.insert(i, nop)
                        i += 1
                        n_split += 1
                    inst.sync_info = mybir.SyncInfo(
                        on_wait=[waits[-1]], on_update=si.on_update
                    )
                i += 1
    return n_split


# --------------------------------------------------------------------------
# device program
# --------------------------------------------------------------------------
def _build_nc(KB, KF, KM, split_waits=True):
    import concourse.bass as bass
    import concourse.mybir as mybir

    fp32 = mybir.dt.float32
    bf16 = mybir.dt.bfloat16
    AF = mybir.ActivationFunctionType
    ALU = mybir.AluOpType
    AX = mybir.AxisListType

    MI = KB // 128  # active-row chunks
    KP = KF + KM  # pre-scaled fg/mask gather width
    SMW = MI + 5

    PatchedTileContext = _make_tile_context_cls()

    nc = bass.Bass("TRN2", target_bir_lowering=False)
    fq_d = nc.declare_dram_parameter("fq", [C, N], bf16, isOutput=False)
    fa_d = nc.declare_dram_parameter("fa", [C, KB], bf16, isOutput=False)
    gm_d = nc.declare_dram_parameter("gm", [C, KP], bf16, isOutput=False)
    gt_d = nc.declare_dram_parameter("gt", [KB, C], bf16, isOutput=False)
    sm_d = nc.declare_dram_parameter("sm", [128, SMW], fp32, isOutput=False)
    wp_d = nc.declare_dram_parameter("wp", [1, KB], bf16, isOutput=False)
    out_d = nc.declare_dram_parameter("out", [2, N], fp32, isOutput=True)

    def nbs(nb):
        return slice(nb * 512, (nb + 1) * 512)

    def c128(cc):
        return slice(cc * 128, (cc + 1) * 128)

    with PatchedTileContext(nc) as tc:
        with (
            tc.tile_pool(name="consts", bufs=1) as consts,
            tc.tile_pool(name="big", bufs=1) as big,
            tc.tile_pool(name="scr", bufs=2) as scr,
            tc.tile_pool(name="small", bufs=1) as small,
        ):
            # ---- input DMAs.  fq ships split by column half so the nb=0
            # pipeline (norms -> cn -> sim) starts while nb=1 still streams.
            # Queue spread: sync = fq first halves; gpsimd = fqa + fq second
            # halves + fgm; scalar = smalls + gt.
            smalls = consts.tile([128, SMW], fp32, tag="smalls")
            nc.scalar.dma_start(smalls, sm_d[:, :])
            wpad = consts.tile([1, KB], bf16, tag="wpad")
            nc.scalar.dma_start(wpad, wp_d[:, :])
            fq = [
                big.tile([128, N], bf16, tag=f"fq{cc}", name=f"fq{cc}")
                for cc in range(CC)
            ]
            for cc in range(CC):
                nc.sync.dma_start(fq[cc][:, 0:512], fq_d[c128(cc), 0:512])
            fqa = []
            for cc in range(CC):
                t = big.tile([128, KB], bf16, tag=f"fqa{cc}", name=f"fqa{cc}")
                nc.gpsimd.dma_start(t, fa_d[c128(cc), :])
                fqa.append(t)
            for cc in range(CC):
                nc.gpsimd.dma_start(fq[cc][:, 512:N], fq_d[c128(cc), 512:N])
            gt = []
            for mi in range(MI):
                t = big.tile([128, C], bf16, tag=f"gt{mi}", name=f"gt{mi}")
                nc.scalar.dma_start(t, gt_d[c128(mi), :])
                gt.append(t)
            fgm = []
            for cc in range(CC):
                t = big.tile([128, KP], bf16, tag=f"fgm{cc}", name=f"fgm{cc}")
                nc.gpsimd.dma_start(t, gm_d[c128(cc), :])
                fgm.append(t)

            # ---- constants
            ones = consts.tile([128, 128], bf16, tag="ones")
            nc.vector.memset(ones, 1.0)
            onef = consts.tile([1, 1], fp32, tag="onef")
            nc.vector.memset(onef, 1.0)
            ones_f = consts.tile([128, 128], fp32, tag="ones_f")
            nc.vector.memset(ones_f, 1.0)
            ln10B = consts.tile([128, 1], fp32, tag="ln10B")
            nc.vector.memset(ln10B, LN10)
            ln2B = consts.tile([1, 1], fp32, tag="ln2B")
            nc.vector.memset(ln2B, LN2)
            # exp bias: {1 -> 0, 0 -> -BIG} from wb-active indicator cols
            biascol = consts.tile([128, MI], fp32, tag="biascol")
            nc.vector.tensor_scalar(
                biascol, smalls[:, 0:MI], BIG, BIG, op0=ALU.mult, op1=ALU.subtract
            )
            wbacb = consts.tile([128, MI], bf16, tag="wbacb")
            nc.vector.tensor_copy(wbacb, smalls[:, 0:MI])
            # preload the activation table while DMAs stream
            dummy = consts.tile([1, 1], fp32, tag="dummy")
            nc.scalar.activation(dummy, onef, AF.Ln)

            rnormB = big.tile([128, N], bf16, tag="rnormB")
            cn = [
                big.tile([128, N], bf16, tag=f"cn{cc}", name=f"cn{cc}")
                for cc in range(CC)
            ]
            scol = small.tile([128, MI], fp32, tag="scol")
            FP1 = small.tile([128, CC], fp32, tag="FP1")
            BGc = small.tile([128, CC], fp32, tag="BGc")
            T = [big.tile([128, N], bf16, tag=f"T{mi}", name=f"T{mi}") for mi in range(MI)]
            T2 = [big.tile([128, N], bf16, tag=f"T2{mi}", name=f"T2{mi}") for mi in range(MI)]
            rcolB = big.tile([128, N], bf16, tag="rcolB")
            BP1 = [big.tile([128, N], bf16, tag=f"BP1{cc}", name=f"BP1{cc}") for cc in range(CC)]
            out0 = big.tile([128, N], fp32, tag="out0")
            out1 = small.tile([1, N], fp32, tag="out1", name="out1")
            FP1s = small.tile([128, CC], bf16, tag="FP1s")

            with tc.tile_pool(name="ps", bufs=1, space="PSUM") as ps:
                # --- active-column norms -> per-partition exp scale column.
                # n2a is row-replicated; Ln/Exp its row, fold the 2x of
                # (2*sim/||fq_k||) via bias=ln2, then a K=1 outer-product
                # matmul turns the row into the [128, MI] column layout.
                n2a = ps.tile([128, KB], fp32, tag="sim", bufs=2, name="n2a")
                for cc in range(CC):
                    sqa_t = scr.tile([128, KB], bf16, tag="sqa", bufs=2, name=f"sqa{cc}")
                    nc.vector.tensor_mul(sqa_t, fqa[cc], fqa[cc])
                    nc.tensor.matmul(n2a, ones, sqa_t, start=(cc == 0), stop=False)
                # pad columns are zero; +1 keeps their rsqrt finite
                nc.tensor.matmul(n2a, ones[0:1, :], wpad, start=False, stop=True)
                lnrow = scr.tile([1, KB], fp32, tag="lnrow", name="lnrow")
                nc.scalar.activation(lnrow, n2a[0:1, :], AF.Ln)
                scrow = scr.tile([1, KB], fp32, tag="scrow", name="scrow")
                nc.scalar.activation(scrow, lnrow, AF.Exp, scale=-0.5, bias=ln2B)
                scolp = ps.tile([128, MI], fp32, tag="bg", bufs=4, name="scolp")
                for mi in range(MI):
                    nc.tensor.matmul(
                        scolp[:, mi : mi + 1],
                        scrow[0:1, mi * 128 : (mi + 1) * 128],
                        onef,
                        start=True,
                        stop=True,
                    )
                nc.vector.tensor_copy(scol, scolp)

                # --- full-feature norms, by column half
                n2ps = [
                    ps.tile([128, 512], fp32, tag="cs", bufs=2, name=f"n2_{nb}")
                    for nb in range(2)
                ]
                for nb in range(2):
                    for cc in range(CC):
                        sq = scr.tile(
                            [128, 512], bf16, tag="sq", bufs=2, name=f"sq{nb}_{cc}"
                        )
                        nc.vector.tensor_mul(
                            sq, fq[cc][:, nbs(nb)], fq[cc][:, nbs(nb)]
                        )
                        nc.tensor.matmul(
                            n2ps[nb], ones, sq, start=(cc == 0), stop=(cc == CC - 1)
                        )
                    tmpn = scr.tile([128, 512], fp32, tag="tmpn", bufs=2, name=f"tn{nb}")
                    nc.scalar.activation(tmpn, n2ps[nb], AF.Ln)
                    nc.scalar.activation(
                        rnormB[:, nbs(nb)], tmpn, AF.Exp, scale=-0.5
                    )
                    for cc in range(CC):
                        nc.vector.tensor_mul(
                            cn[cc][:, nbs(nb)], fq[cc][:, nbs(nb)], rnormB[:, nbs(nb)]
                        )

                # --- gram + exp + colsum, one 512-col group at a time; the
                # lhsT is the raw gathered fqa, its normalization applied via
                # the per-partition activation scale
                for nb in range(2):
                    cs_t = ps.tile([128, 512], fp32, tag="cs", bufs=2, name=f"cs{nb}")
                    for mi in range(MI):
                        simp = ps.tile(
                            [128, 512], fp32, tag="sim", bufs=2, name=f"sim{nb}_{mi}"
                        )
                        for cc in range(CC):
                            nc.tensor.matmul(
                                simp,
                                fqa[cc][:, mi * 128 : (mi + 1) * 128],
                                cn[cc][:, nbs(nb)],
                                start=(cc == 0),
                                stop=(cc == CC - 1),
                            )
                        nc.scalar.activation(
                            T[mi][:, nbs(nb)],
                            simp,
                            AF.Exp,
                            bias=biascol[:, mi : mi + 1],
                            scale=scol[:, mi : mi + 1],
                        )
                        nc.tensor.matmul(
                            cs_t,
                            ones,
                            T[mi][:, nbs(nb)],
                            start=(mi == 0),
                            stop=(mi == MI - 1),
                        )
                    # softmax denominators: rcol = exp(-ln(colsum))
                    tcs = scr.tile([128, 512], fp32, tag="tcs", bufs=2, name=f"tcs{nb}")
                    nc.scalar.activation(tcs, cs_t, AF.Ln)
                    nc.scalar.activation(rcolB[:, nbs(nb)], tcs, AF.Exp, scale=-1.0)

                # --- bg prototype via gt: row = wba^T gt, then K=1 outer
                # matmuls to the [128, CC] column layout
                bgrow_p = ps.tile([1, C], fp32, tag="bg", bufs=4, name="bgrow")
                for mi in range(MI):
                    nc.tensor.matmul(
                        bgrow_p,
                        wbacb[:, mi : mi + 1],
                        gt[mi],
                        start=(mi == 0),
                        stop=(mi == MI - 1),
                    )
                bgrow = scr.tile([1, C], fp32, tag="bgrow_s", name="bgrow_s")
                nc.vector.tensor_copy(bgrow, bgrow_p)
                bgcolp = ps.tile([128, CC], fp32, tag="bg", bufs=4, name="bgcolp")
                for cc in range(CC):
                    nc.tensor.matmul(
                        bgcolp[:, cc : cc + 1],
                        bgrow[0:1, c128(cc)],
                        onef,
                        start=True,
                        stop=True,
                    )
                nc.vector.tensor_scalar_mul(BGc, bgcolp, smalls[:, MI : MI + 1])

                # --- fg prototype: single pre-scaled segment sum per chunk
                for cc in range(CC):
                    nc.vector.reduce_sum(FP1[:, cc : cc + 1], fgm[cc], axis=AX.X)
                sqf = small.tile([128, CC], fp32, tag="sqf")
                nc.vector.tensor_mul(sqf, FP1, FP1)
                rsf = small.tile([128, 1], fp32, tag="rsf")
                nc.vector.reduce_sum(rsf, sqf, axis=AX.X)
                nfp_ps = ps.tile([1, 1], fp32, tag="cs", bufs=2, name="nfp")
                nc.tensor.matmul(nfp_ps, ones_f[:, 0:1], rsf, start=True, stop=True)
                nfp_sb = small.tile([1, 1], fp32, tag="nfp_sb")
                nc.vector.tensor_copy(nfp_sb, nfp_ps)
                f10_ps = ps.tile([128, 1], fp32, tag="cs", bufs=2, name="f10p")
                nc.tensor.matmul(f10_ps, ones_f[0:1, :], nfp_sb, start=True, stop=True)
                f10a = small.tile([128, 1], fp32, tag="f10a")
                nc.scalar.activation(f10a, f10_ps, AF.Ln)
                f10B = small.tile([128, 1], fp32, tag="f10B")
                nc.scalar.activation(f10B, f10a, AF.Exp, scale=-0.5, bias=ln10B)
                nc.vector.tensor_scalar_mul(FP1s, FP1, f10B)

                # --- softmax renorm folded into T
                for mi in range(MI):
                    nc.vector.tensor_mul(T2[mi], T[mi], rcolB)

                # --- recon nb0; BP1 nb0; p/q nb0 overlap recon nb1
                bg = {}
                for nb in range(2):
                    for cc in range(CC):
                        bg[nb, cc] = ps.tile(
                            [128, 512], fp32, tag="bg", bufs=4, name=f"bg{nb}_{cc}"
                        )
                us = [ps.tile([128, 512], fp32, tag="sim", bufs=2, name=f"us{nb}") for nb in range(2)]
                qs = [ps.tile([128, 512], fp32, tag="cs", bufs=2, name=f"qs{nb}") for nb in range(2)]

                def recon(nb):
                    for mi in range(MI):
                        for cc in range(CC):
                            nc.tensor.matmul(
                                bg[nb, cc],
                                gt[mi][:, c128(cc)],
                                T2[mi][:, nbs(nb)],
                                start=(mi == 0),
                                stop=(mi == MI - 1),
                            )

                def bp1_pq(nb):
                    for cc in range(CC):
                        if cc < 2:
                            nc.scalar.activation(
                                BP1[cc][:, nbs(nb)],
                                bg[nb, cc],
                                AF.Identity,
                                bias=BGc[:, cc : cc + 1],
                            )
                        else:
                            nc.vector.tensor_scalar_add(
                                BP1[cc][:, nbs(nb)], bg[nb, cc], BGc[:, cc : cc + 1]
                            )
                    for cc in range(CC):
                        p_t = scr.tile(
                            [128, 512], bf16, tag="p", bufs=2, name=f"p{nb}_{cc}"
                        )
                        nc.vector.tensor_mul(p_t, cn[cc][:, nbs(nb)], BP1[cc][:, nbs(nb)])
                        q_t = scr.tile(
                            [128, 512], bf16, tag="q", bufs=2, name=f"q{nb}_{cc}"
                        )
                        nc.vector.tensor_mul(q_t, BP1[cc][:, nbs(nb)], BP1[cc][:, nbs(nb)])
                        nc.tensor.matmul(
                            us[nb], ones, p_t, start=(cc == 0), stop=(cc == CC - 1)
                        )
                        nc.tensor.matmul(
                            qs[nb], ones, q_t, start=(cc == 0), stop=(cc == CC - 1)
                        )

                def finish(nb):
                    trq = scr.tile([128, 512], fp32, tag="trq", bufs=2, name=f"trq{nb}")
                    nc.scalar.activation(trq, qs[nb], AF.Ln)
                    r1 = scr.tile([128, 512], fp32, tag="r1", bufs=2, name=f"r1{nb}")
                    nc.scalar.activation(r1, trq, AF.Exp, scale=-0.5, bias=ln10B)
                    nc.vector.tensor_mul(out0[:, nbs(nb)], us[nb], r1)
                    nc.sync.dma_start(out_d[0:1, nbs(nb)], out0[0:1, nbs(nb)])

                recon(0)
                # fg similarity rows fill the PE gap before recon nb1
                dfg = []
                for nb in range(2):
                    d_t = ps.tile([1, 512], fp32, tag="cs", bufs=2, name=f"dfg{nb}")
                    for cc in range(CC):
                        nc.tensor.matmul(
                            d_t,
                            FP1s[:, cc : cc + 1],
                            cn[cc][:, nbs(nb)],
                            start=(cc == 0),
                            stop=(cc == CC - 1),
                        )
                    dfg.append(d_t)
                bp1_pq(0)
                for nb in range(2):
                    nc.scalar.copy(out1[:, nbs(nb)], dfg[nb])
                nc.sync.dma_start(out_d[1:2, :], out1)
                recon(1)
                finish(0)
                bp1_pq(1)
                finish(1)

    if split_waits:
        _split_multi_waits(nc)
    return nc


def _get_nc(KB, KF, KM):
    key = (KB, KF, KM)
    if key not in _cache:
        _cache[key] = _build_nc(KB, KF, KM)
    return _cache[key]


# --------------------------------------------------------------------------
# host prep: gathers + scalars
# --------------------------------------------------------------------------
def _round_up(x, m):
    return ((x + m - 1) // m) * m


def _make_in_maps(feature_q, support_feat, support_mask):
    wf, wb = _host_select_weights(feature_q, support_feat, support_mask)
    fqr = feature_q.reshape(B, C, N).astype(ml_dtypes.bfloat16)
    sfr = support_feat.reshape(B, C, N).astype(ml_dtypes.bfloat16)
    mfr = support_mask.reshape(B, N) == 1

    nb_ = wb.sum(1).astype(int)
    nf_ = wf.sum(1).astype(int)
    nm_ = mfr.sum(1).astype(int)
    KB = max(KB0, _round_up(nb_.max() + 1, 128))
    KF = max(KF0, _round_up(nf_.max(), 64))
    KM = max(KM0, _round_up(max(nm_.max(), 1), 64))
    MI = KB // 128

    in_maps = []
    for b in range(B):
        ib = np.where(wb[b] > 0)[0]
        iff = np.where(wf[b] > 0)[0]
        im = np.where(mfr[b])[0]
        rcf = np.float32(1.0 / max(nf_[b], 1))
        rcm = np.float32(1.0 / (nm_[b] + 1e-5))
        fa = np.zeros((C, KB), ml_dtypes.bfloat16)
        fa[:, : len(ib)] = fqr[b][:, ib]
        gm = np.zeros((C, KF + KM), ml_dtypes.bfloat16)
        gm[:, : len(iff)] = (
            fqr[b][:, iff].astype(np.float32) * rcf
        ).astype(ml_dtypes.bfloat16)
        gm[:, KF : KF + len(im)] = (
            sfr[b][:, im].astype(np.float32) * rcm
        ).astype(ml_dtypes.bfloat16)
        gt = np.zeros((KB, C), ml_dtypes.bfloat16)
        gt[: len(ib)] = fqr[b][:, ib].T
        wba = np.zeros(KB, np.float32)
        wba[: len(ib)] = 1.0
        sm = np.zeros((128, MI + 5), np.float32)
        sm[:, 0:MI] = wba.reshape(MI, 128).T
        sm[:, MI] = (3.0 / 7.0) / max(nb_[b], 1)
        wp = (1.0 - wba).astype(ml_dtypes.bfloat16)[None, :]
        in_maps.append(
            {"fq": fqr[b], "fa": fa, "gm": gm, "gt": gt, "sm": sm, "wp": wp}
        )
    return in_maps, (KB, KF, KM)


def run_sharded(feature_q, support_feat, support_mask, **kwargs):
    """Run on all 8 cores; returns (output [B,2,H,W], BassKernelResults)."""
    from concourse.bass_utils import run_bass_kernel_spmd

    in_maps, caps = _make_in_maps(
        np.asarray(feature_q), np.asarray(support_feat), np.asarray(support_mask)
    )
    nc = _get_nc(*caps)
    res = run_bass_kernel_spmd(nc, in_maps, core_ids=list(range(B)), **kwargs)
    out = np.stack([res.results[b]["out"] for b in range(B)])
    return out.reshape(B, 2, H, W).astype(np.float32), res


def kernel(feature_q, support_feat, support_mask):
    out, _ = run_sharded(
        np.asarray(feature_q), np.asarray(support_feat), np.asarray(support_mask)
    )
    return out
